# revision 1
# baseline (speedup 1.0000x reference)
"""AdaAttN Trainium2 kernel — 8-core SPMD, data-parallel over (batch, query-half).

Each core handles one (batch b, query half): 2048 of the 4096 query positions.
Transpose-free layout throughout:

  F  = f_w @ content_key[b][:, q]        [ck, q]   (f32 matmul)
  G  = g_w @ style_key[b]                [ck, k]   (f32 matmul)
  HT = (h_w @ style[b]).T                [k, c]    (fp32r matmul, computed
        directly with lhsT=style, rhs=h_w.T — no transposes anywhere)
  S^T[k, q] = G.T @ F                    (bf16 two-term-split: ~f32-exact)
  P^T = exp(S^T - 120)                   (constant safe shift; logits max ~190,
                                          min per-query max ~80, window [103,167])
  mean/second/l = (HT | HT^2 | 1).T @ P^T  (bf16 two-term-split matmuls)
  out = sqrt(relu(second/l - (mean/l)^2)) * mvnorm(content) + mean/l + h_b

PSUM discipline: every matmul accumulation group owns a full 2KB bank (a
start=True matmul clears has_written for its whole bank).  Per query block:
pass A accumulates mean (4 banks) + l (1 bank) while the S^T ring uses the
remaining 3; mean is folded to SBUF between passes; pass B accumulates the
second moment (4 banks).  The P^T bf16 split pair is stored per block to feed
both passes.  h_b is folded into the final add (variance is shift-invariant);
f_b/g_b are added at F/G PSUM evacuation.
"""

import numpy as np

import concourse.bass as bass
import concourse.mybir as mybir
from concourse import bacc
from concourse.bass import ts
from concourse.bass_utils import run_bass_kernel_spmd
from concourse.tile import TileContext

F32 = mybir.dt.float32
F32R = mybir.dt.float32r
BF16 = mybir.dt.bfloat16
AF = mybir.ActivationFunctionType
ALU = mybir.AluOpType

B, C, HW = 4, 512, 4096  # batch, channels (=key planes), spatial
Q = 2048                 # queries per core (half a batch)
QB = 512                 # query block
QH = 256                 # half-block (DMA/staging granularity)
NBLK = Q // QB           # 8
CC = C // 128            # 4 channel chunks
NKT = HW // 128          # 32 key tiles
NNB = HW // 512          # 8 key 512-blocks
SHIFT = 120.0
EPS = 1e-5


def _build():
    nc = bacc.Bacc("TRN2", target_bir_lowering=False, debug=False)

    ckq = nc.declare_dram_parameter("ckq", [C, Q], F32, isOutput=False)
    sk = nc.declare_dram_parameter("sk", [C, HW], F32, isOutput=False)
    st = nc.declare_dram_parameter("st", [C, HW], F32, isOutput=False)
    ct = nc.declare_dram_parameter("ct", [C, HW], F32, isOutput=False)
    ctq = nc.declare_dram_parameter("ctq", [C, Q], F32, isOutput=False)
    fwT1 = nc.declare_dram_parameter("fwT1", [C, C], BF16, isOutput=False)
    fwT2 = nc.declare_dram_parameter("fwT2", [C, C], BF16, isOutput=False)
    gwT1 = nc.declare_dram_parameter("gwT1", [C, C], BF16, isOutput=False)
    gwT2 = nc.declare_dram_parameter("gwT2", [C, C], BF16, isOutput=False)
    hwT = nc.declare_dram_parameter("hwT", [C, C], F32, isOutput=False)
    fb = nc.declare_dram_parameter("fb", [C, 1], F32, isOutput=False)
    gb = nc.declare_dram_parameter("gb", [C, 1], F32, isOutput=False)
    hb = nc.declare_dram_parameter("hb", [C, 1], F32, isOutput=False)
    out = nc.declare_dram_parameter("out", [C, Q], F32, isOutput=True)

    # [512, M] dram -> [128, 4, M] (partition = channel-within-chunk)
    def chunked(ap):
        return ap.rearrange("(a p) m -> p a m", p=128)

    with TileContext(nc) as tc:
        with (
            tc.tile_pool(name="const", bufs=1) as const,
            tc.tile_pool(name="stage", bufs=3) as stage,
            tc.tile_pool(name="big", bufs=1) as big,
            tc.tile_pool(name="work", bufs=2) as work,
            tc.tile_pool(name="scratch", bufs=1) as scratch,
            tc.tile_pool(name="psacc", bufs=4, space="PSUM") as psacc,
            tc.tile_pool(name="psmm", bufs=4, space="PSUM") as psmm,
        ):
            # ---------------- constants ----------------
            fwT1_sb = const.tile([128, CC, C], BF16)
            nc.sync.dma_start(out=fwT1_sb, in_=chunked(fwT1.ap()))
            fwT2_sb = const.tile([128, CC, C], BF16)
            nc.sync.dma_start(out=fwT2_sb, in_=chunked(fwT2.ap()))
            fb_sb = const.tile([128, CC, 1], F32)
            gb_sb = const.tile([128, CC, 1], F32)
            hb_sb = const.tile([128, CC, 1], F32)
            nc.sync.dma_start(out=fb_sb, in_=chunked(fb.ap()))
            nc.sync.dma_start(out=gb_sb, in_=chunked(gb.ap()))
            nc.sync.dma_start(out=hb_sb, in_=chunked(hb.ap()))
            negshift = const.tile([128, 1], F32)
            nc.vector.memset(negshift, -SHIFT)
            ones_f = const.tile([128, 1], F32)
            nc.vector.memset(ones_f, 1.0)
            cmean = const.tile([128, CC, 1], F32)
            crstd = const.tile([128, CC, 1], F32)
            crstd2 = const.tile([128, CC, 1], F32)

            # ---------------- G = g_w @ style_key (f32) -> bf16 split pair ----------------
            G1 = big.tile([128, CC, HW], BF16)
            G2 = big.tile([128, CC, HW], BF16)
            sk_ch = chunked(sk.ap())
            gwT1_sb = const.tile([128, CC, C], BF16)
            nc.sync.dma_start(out=gwT1_sb, in_=chunked(gwT1.ap()))
            gwT2_sb = const.tile([128, CC, C], BF16)
            nc.sync.dma_start(out=gwT2_sb, in_=chunked(gwT2.ap()))
            for nb in range(2 * NNB):
                sk_t = stage.tile([128, CC, 256], F32, tag="stage")
                nc.sync.dma_start(out=sk_t, in_=sk_ch[:, :, ts(nb, 256)])
                sk1 = work.tile([128, CC, 256], BF16, tag="sk1", name=f"sk1_{nb}")
                nc.scalar.activation(sk1, sk_t, AF.Copy, bias=0.0, scale=1.0)
                sk2 = work.tile([128, CC, 256], BF16, tag="sk2", name=f"sk2_{nb}")
                nc.vector.tensor_sub(sk2, sk_t, sk1)
                for co in range(CC):
                    gps = psmm.tile([128, 256], F32, tag="mm")
                    for ci in range(CC):
                        w1 = gwT1_sb[:, ci, ts(co, 128)]
                        nc.tensor.matmul(
                            gps, w1, sk1[:, ci, :], start=(ci == 0), stop=False
                        )
                        nc.tensor.matmul(gps, w1, sk2[:, ci, :], start=False, stop=False)
                        nc.tensor.matmul(
                            gps,
                            gwT2_sb[:, ci, ts(co, 128)],
                            sk1[:, ci, :],
                            start=False,
                            stop=(ci == CC - 1),
                        )
                    g1s = G1[:, co, ts(nb, 256)]
                    nc.vector.tensor_scalar_add(g1s, gps, gb_sb[:, co, :])
                    nc.vector.scalar_tensor_tensor(
                        out=G2[:, co, ts(nb, 256)],
                        in0=gps,
                        scalar=gb_sb[:, co, :],
                        in1=g1s,
                        op0=ALU.add,
                        op1=ALU.subtract,
                    )

            # ---------------- HT[k, c] = (h_w @ style).T (fp32r) -> bf16 split ----------------
            HTF = big.tile([128, NKT, C], mybir.dt.float16)
            st_ch = chunked(st.ap()).bitcast(F32R)
            hwT_sb = stage.tile([128, CC, C], F32R, tag="wslot", bufs=1)
            nc.sync.dma_start(out=hwT_sb, in_=chunked(hwT.ap()).bitcast(F32R))
            for nb in range(2 * NNB):
                st_t = stage.tile([128, CC, 256], F32R, tag="stage")
                nc.sync.dma_start(out=st_t, in_=st_ch[:, :, ts(nb, 256)])
                for w in range(2):
                    kt = nb * 2 + w
                    hps = psmm.tile([128, 512], F32, tag="mm")
                    for ci in range(CC):
                        nc.tensor.matmul(
                            hps,
                            st_t[:, ci, ts(w, 128)],
                            hwT_sb[:, ci, :],
                            start=(ci == 0),
                            stop=(ci == CC - 1),
                        )
                    nc.scalar.activation(
                        HTF[:, kt, :], hps, AF.Copy, bias=0.0, scale=1.0
                    )

            # ---------------- main loop over query blocks ----------------
            ckq_ch = chunked(ckq.ap())
            ctq_ch = chunked(ctq.ap())
            out_ch = chunked(out.ap())
            ct_ch = chunked(ct.ap())
            stats_all = scratch.tile([128, 4, 8, 6], F32, tag="bnstats")

            def emit_stats_piece(i):
                # piece i: cc = i // 4, quarter = i % 4  -> one DMA + 2 bn_stats
                cc, quart = i // 4, i % 4
                ctp = stage.tile(
                    [128, 2, 512], F32, tag="stage", name=f"ctp{i}"
                )
                nc.sync.dma_start(
                    out=ctp,
                    in_=ct_ch[:, cc, ts(quart, 1024)].rearrange(
                        "p (a m) -> p a m", a=2
                    ),
                )
                for g in range(2):
                    nc.vector.bn_stats(
                        out=stats_all[:, cc, quart * 2 + g, :], in_=ctp[:, g, :]
                    )

            def emit_stats_tail():
                for cc in range(CC):
                    mv = scratch.tile([128, 2], F32, tag="bnmv")
                    nc.vector.bn_aggr(
                        out=mv,
                        in_=stats_all[:, cc, :, :].rearrange("p a b -> p (a b)"),
                    )
                    nc.vector.tensor_copy(cmean[:, cc, :], mv[:, 0:1])
                    tv = scratch.tile([128, 1], F32, tag="bntv")
                    nc.vector.tensor_scalar(
                        out=tv,
                        in0=mv[:, 1:2],
                        scalar1=float(HW) / float(HW - 1),
                        scalar2=EPS,
                        op0=ALU.mult,
                        op1=ALU.add,
                    )
                    sq = scratch.tile([128, 1], F32, tag="bnsq")
                    nc.scalar.activation(sq, tv, AF.Sqrt, bias=0.0, scale=1.0)
                    nc.vector.reciprocal(crstd[:, cc, :], sq)
                    nc.vector.tensor_mul(
                        crstd2[:, cc, :], crstd[:, cc, :], crstd[:, cc, :]
                    )

            for blk in range(NBLK):
                # F block -> bf16 split pair (two half-blocks for staging)
                F1 = work.tile([128, CC, QB], BF16, tag="f1")
                F2 = work.tile([128, CC, QB], BF16, tag="f2")
                for hh in range(2):
                    qoff = blk * QB + hh * QH
                    ckq_t = stage.tile([128, CC, QH], F32, tag="stage")
                    nc.sync.dma_start(
                        out=ckq_t, in_=ckq_ch[:, :, qoff : qoff + QH]
                    )
                    ck1 = work.tile(
                        [128, CC, QH], BF16, tag="sk1", name=f"ck1_{blk}_{hh}"
                    )
                    nc.scalar.activation(ck1, ckq_t, AF.Copy, bias=0.0, scale=1.0)
                    ck2 = work.tile(
                        [128, CC, QH], BF16, tag="sk2", name=f"ck2_{blk}_{hh}"
                    )
                    nc.vector.tensor_sub(ck2, ckq_t, ck1)
                    hs = slice(hh * QH, (hh + 1) * QH)
                    for co in range(CC):
                        fps = psmm.tile([128, QH], F32, tag="mm")
                        for ci in range(CC):
                            w1 = fwT1_sb[:, ci, ts(co, 128)]
                            nc.tensor.matmul(
                                fps, w1, ck1[:, ci, :], start=(ci == 0), stop=False
                            )
                            nc.tensor.matmul(
                                fps, w1, ck2[:, ci, :], start=False, stop=False
                            )
                            nc.tensor.matmul(
                                fps,
                                fwT2_sb[:, ci, ts(co, 128)],
                                ck1[:, ci, :],
                                start=False,
                                stop=(ci == CC - 1),
                            )
                        f1s = F1[:, co, hs]
                        nc.vector.tensor_scalar_add(f1s, fps, fb_sb[:, co, :])
                        nc.vector.scalar_tensor_tensor(
                            out=F2[:, co, hs],
                            in0=fps,
                            scalar=fb_sb[:, co, :],
                            in1=f1s,
                            op0=ALU.add,
                            op1=ALU.subtract,
                        )

                mean_ps = [
                    psacc.tile([128, QB], F32, tag="acc", name=f"mean{i}")
                    for i in range(CC)
                ]
                sacc = work.tile([128, CC, QB], F32, tag="sacc", bufs=1)
                l_part = work.tile([128, QB], F32, tag="lpart", bufs=1)
                pts = {}

                def emit_st(kt):
                    sps = psmm.tile([128, QB], F32, tag="mm", name=f"sps{kt}")
                    for ci in range(CC):
                        g1s = G1[:, ci, ts(kt, 128)]
                        nc.tensor.matmul(
                            sps, g1s, F1[:, ci, :], start=(ci == 0), stop=False
                        )
                        nc.tensor.matmul(sps, g1s, F2[:, ci, :], start=False, stop=False)
                        nc.tensor.matmul(
                            sps,
                            G2[:, ci, ts(kt, 128)],
                            F1[:, ci, :],
                            start=False,
                            stop=(ci == CC - 1),
                        )
                    ptf = work.tile([128, QB], F32, tag="ptf", name=f"ptf{kt}")
                    nc.scalar.activation(ptf, sps, AF.Exp, bias=negshift, scale=1.0)
                    pt1 = work.tile([128, QB], BF16, tag="pt1", name=f"pt1_{kt}", bufs=3)
                    nc.scalar.activation(pt1, ptf, AF.Copy, bias=0.0, scale=1.0)
                    pt2 = work.tile([128, QB], BF16, tag="pt2", name=f"pt2_{kt}", bufs=3)
                    nc.vector.tensor_sub(pt2, ptf, pt1)
                    if kt == 0:
                        nc.vector.tensor_copy(l_part, ptf)
                    else:
                        nc.vector.tensor_add(l_part, l_part, ptf)
                    pts[kt] = (pt1, pt2)

                def emit_av(kt):
                    first, last = kt == 0, kt == NKT - 1
                    pt1, pt2 = pts[kt]
                    for cc in range(CC):
                        m1 = HTF[:, kt, ts(cc, 128)]
                        nc.tensor.matmul(mean_ps[cc], m1, pt1, start=first, stop=False)
                        nc.tensor.matmul(mean_ps[cc], m1, pt2, start=False, stop=last)

                def emit_h2(kt):
                    h2f = work.tile(
                        [128, C], F32, tag="h2f", name=f"h2f{kt}", bufs=1
                    )
                    nc.scalar.activation(
                        h2f, HTF[:, kt, :], AF.Square, bias=0.0, scale=1.0
                    )
                    h2a = work.tile([128, C], BF16, tag="h2a", name=f"h2a{kt}")
                    nc.scalar.activation(h2a, h2f, AF.Copy, bias=0.0, scale=1.0)
                    h2b = work.tile([128, C], BF16, tag="h2b", name=f"h2b{kt}")
                    nc.vector.tensor_sub(h2b, h2f, h2a)
                    return (h2a, h2b)

                def emit_sec(kt, h2t):
                    h2a, h2b = h2t
                    pt1, pt2 = pts.pop(kt)
                    for cc in range(CC):
                        s1 = h2a[:, ts(cc, 128)]
                        s2 = h2b[:, ts(cc, 128)]
                        sp = psmm.tile(
                            [128, QB], F32, tag="mm", name=f"secp{kt}_{cc}"
                        )
                        nc.tensor.matmul(sp, s1, pt1, start=True, stop=False)
                        nc.tensor.matmul(sp, s1, pt2, start=False, stop=False)
                        nc.tensor.matmul(sp, s2, pt1, start=False, stop=True)
                        if kt == 0:
                            nc.vector.tensor_copy(sacc[:, cc, :], sp)
                        else:
                            nc.vector.tensor_add(
                                sacc[:, cc, :], sacc[:, cc, :], sp
                            )

                emit_st(0)
                if blk == 0:
                    emit_stats_piece(0)
                for kt in range(1, NKT):
                    emit_st(kt)
                    if blk == 0 and kt < 16:
                        emit_stats_piece(kt)
                    emit_av(kt - 1)
                    h2t = emit_h2(kt - 1)
                    emit_sec(kt - 1, h2t)
                if blk == 0:
                    emit_stats_tail()
                l_ps = psmm.tile([1, QB], F32, tag="mm", name="lps")
                nc.tensor.matmul(l_ps, ones_f, l_part, start=True, stop=True)
                rinv = scratch.tile([1, QB], F32, tag="ptmp")
                nc.vector.reciprocal(rinv, l_ps)
                rbc = scratch.tile([128, QB], F32, tag="rbc")
                nc.gpsimd.partition_broadcast(rbc, rinv[:1, :])
                emit_av(NKT - 1)
                h2t = emit_h2(NKT - 1)
                emit_sec(NKT - 1, h2t)

                # ---- post: l, variance, std, assemble output ----
                # evacuate mean accumulators immediately (ACT) to free PSUM
                macc = work.tile([128, CC, QB], F32, tag="macc", bufs=1)
                for cc in range(CC):
                    nc.scalar.activation(
                        macc[:, cc, :], mean_ps[cc], AF.Copy, bias=0.0, scale=1.0
                    )
                ct_p0 = stage.tile([128, CC, QH], F32, tag="stage", name="ctp0")
                nc.sync.dma_start(
                    out=ct_p0, in_=ctq_ch[:, :, blk * QB : blk * QB + QH]
                )
                ct_p1 = stage.tile([128, CC, QH], F32, tag="stage", name="ctp1")
                nc.sync.dma_start(
                    out=ct_p1, in_=ctq_ch[:, :, blk * QB + QH : (blk + 1) * QB]
                )
                for cc in range(CC):
                    out_sb = work.tile([128, QB], F32, tag="outb")
                    mnp_t = work.tile([128, QB], F32, tag="lpart", bufs=1, name=f"mnpt{cc}")
                    nc.vector.tensor_mul(mnp_t, macc[:, cc, :], rbc)
                    e2 = scratch.tile([128, QB], F32, tag="ptmp")
                    nc.vector.tensor_mul(e2, sacc[:, cc, :], rbc)
                    msq = work.tile([128, QB], F32, tag="outb", name=f"msq{cc}")
                    nc.scalar.activation(msq, mnp_t, AF.Square, bias=0.0, scale=1.0)
                    var = work.tile([128, QB], F32, tag="ptf")
                    nc.vector.tensor_sub(var, e2, msq)
                    vmx = scratch.tile([128, QB], F32, tag="ptmp")
                    nc.scalar.activation(vmx, var, AF.Relu, bias=0.0, scale=1.0)
                    stdt = work.tile([128, QB], F32, tag="ptf", name=f"stdt{cc}")
                    nc.scalar.activation(
                        stdt, vmx, AF.Sqrt, bias=0.0, scale=crstd2[:, cc, :]
                    )
                    o1 = scratch.tile([128, QB], F32, tag="ptmp")
                    for hh, ctp in ((0, ct_p0), (1, ct_p1)):
                        nc.vector.scalar_tensor_tensor(
                            out=o1[:, ts(hh, QH)],
                            in0=ctp[:, cc, :],
                            scalar=cmean[:, cc, :],
                            in1=stdt[:, ts(hh, QH)],
                            op0=ALU.subtract,
                            op1=ALU.mult,
                        )
                    nc.vector.scalar_tensor_tensor(
                        out=out_sb,
                        in0=mnp_t,
                        scalar=hb_sb[:, cc, :],
                        in1=o1,
                        op0=ALU.add,
                        op1=ALU.add,
                    )
                    nc.sync.dma_start(
                        out=out_ch[:, cc, ts(blk, QB)], in_=out_sb
                    )

    nc.compile()
    return nc


_NC_CACHE = []


def kernel(content, style, content_key, style_key, f_w, f_b, g_w, g_b, h_w, h_b):
    if not _NC_CACHE:
        _NC_CACHE.append(_build())
    nc = _NC_CACHE[0]

    import ml_dtypes

    c32 = lambda a: np.ascontiguousarray(a, dtype=np.float32)

    def bsplit(w):
        w = np.asarray(w, np.float32)
        w1 = w.astype(ml_dtypes.bfloat16)
        w2 = (w - w1.astype(np.float32)).astype(ml_dtypes.bfloat16)
        return np.ascontiguousarray(w1), np.ascontiguousarray(w2)

    fwT1, fwT2 = bsplit(f_w.T)
    gwT1, gwT2 = bsplit(g_w.T)
    hwT = c32(h_w.T)
    fbr = c32(np.asarray(f_b).reshape(C, 1))
    gbr = c32(np.asarray(g_b).reshape(C, 1))
    hbr = c32(np.asarray(h_b).reshape(C, 1))

    in_maps = []
    for core in range(8):
        b, h = core // 2, core % 2
        qsl = slice(h * Q, (h + 1) * Q)
        in_maps.append(
            {
                "ckq": c32(np.asarray(content_key[b]).reshape(C, HW)[:, qsl]),
                "sk": c32(np.asarray(style_key[b]).reshape(C, HW)),
                "st": c32(np.asarray(style[b]).reshape(C, HW)),
                "ct": c32(np.asarray(content[b]).reshape(C, HW)),
                "ctq": c32(np.asarray(content[b]).reshape(C, HW)[:, qsl]),
                "fwT1": fwT1,
                "fwT2": fwT2,
                "gwT1": gwT1,
                "gwT2": gwT2,
                "hwT": hwT,
                "fb": fbr,
                "gb": gbr,
                "hb": hbr,
            }
        )

    res = run_bass_kernel_spmd(nc, in_maps, core_ids=list(range(8)), trace=True)
    kernel.last_exec_time_ns = res.exec_time_ns

    full = np.empty((B, C, HW), dtype=np.float32)
    for core in range(8):
        b, h = core // 2, core % 2
        full[b][:, h * Q : (h + 1) * Q] = res.results[core]["out"]
    return full.reshape(B, C, 64, 64)


kernel.last_exec_time_ns = None



# revision 11
# speedup vs baseline: 1.4095x; 1.4095x over previous
"""AdaAttN Trainium2 kernel — 8-core SPMD, data-parallel over (batch, query-half).

Each core handles one (batch b, query half): 2048 of the 4096 query positions.
Single-matmul precision strategy (no bf16 two-term splits): the tensor engine
runs fp32r (moving free-dim >= 256) and fp16 matmuls at the same 1 cycle/row
rate as bf16, with ~11-bit-mantissa operand precision (FP22 internal), so:

  F  = f_w @ content_key[b][:, q]   [ck, q]  f32r matmul -> fp16
  G  = g_w @ style_key[b]           [ck, k]  f32r matmul -> fp16
  HT = (h_w @ style[b]).T           [k, c]   f32r matmul -> fp16 (HTF)
  S^T[k, q] = G.T @ F                        fp16 x fp16 matmul (4 MMs/kt)
  P = exp(S^T - 120) -> bf16 (pblk), stored for the whole query block

Consistency discipline for the variance: the bf16 P values are the single
source of truth — the normalizer l = sum_k P (DVE, from the same bf16
values), mean = HTF.T @ P, second = (HTF^2).T @ P with HTF^2 held as an
exact bf16 pair (h2a + h2b).  Then second/l - (mean/l)^2 is the exact
variance of quantized values under a genuine probability distribution:
nonnegative, no catastrophic-cancellation amplification of quantization
noise.  Per query block: pass A accumulates mean in 4 PSUM banks while the
S ring uses the other 4; mean is folded to SBUF; pass B accumulates second
in the freed banks reusing the stored P block.  h2a/h2b are precomputed
once per core.  h_b is folded into the final add (variance is
shift-invariant); f_b/g_b are added at F/G PSUM evacuation.

  out = sqrt(relu(second/l - (mean/l)^2)) * mvnorm(content) + mean/l + h_b
"""

import numpy as np

import concourse.bass as bass
import concourse.mybir as mybir
from concourse import bacc
from concourse.bass import ts
from concourse.bass_utils import run_bass_kernel_spmd
from concourse.tile import TileContext

F32 = mybir.dt.float32
F32R = mybir.dt.float32r
F16 = mybir.dt.float16
BF16 = mybir.dt.bfloat16
AF = mybir.ActivationFunctionType
ALU = mybir.AluOpType

B, C, HW = 4, 512, 4096  # batch, channels (=key planes), spatial
Q = 2048                 # queries per core (half a batch)
QB = 512                 # query block
QH = 256                 # half-block (DMA/staging granularity)
NBLK = Q // QB           # 4
CC = C // 128            # 4 channel chunks
NKT = HW // 128          # 32 key tiles
SHIFT = 120.0
EPS = 1e-5


def _build():
    nc = bacc.Bacc("TRN2", target_bir_lowering=False, debug=False)

    ckq = nc.declare_dram_parameter("ckq", [C, Q], F32, isOutput=False)
    sk = nc.declare_dram_parameter("sk", [C, HW], F32, isOutput=False)
    st = nc.declare_dram_parameter("st", [C, HW], F32, isOutput=False)
    ct = nc.declare_dram_parameter("ct", [C, HW], F32, isOutput=False)
    ctq = nc.declare_dram_parameter("ctq", [C, Q], F32, isOutput=False)
    fwT = nc.declare_dram_parameter("fwT", [C, C], F32, isOutput=False)
    gwT = nc.declare_dram_parameter("gwT", [C, C], F32, isOutput=False)
    hwT = nc.declare_dram_parameter("hwT", [C, C], F32, isOutput=False)
    fb = nc.declare_dram_parameter("fb", [C, 1], F32, isOutput=False)
    gb = nc.declare_dram_parameter("gb", [C, 1], F32, isOutput=False)
    hb = nc.declare_dram_parameter("hb", [C, 1], F32, isOutput=False)
    out = nc.declare_dram_parameter("out", [C, Q], F32, isOutput=True)

    # [512, M] dram -> [128, 4, M] (partition = channel-within-chunk)
    def chunked(ap):
        return ap.rearrange("(a p) m -> p a m", p=128)

    with TileContext(nc) as tc:
        with (
            tc.tile_pool(name="const", bufs=1) as const,
            tc.tile_pool(name="stage", bufs=2) as stage,
            tc.tile_pool(name="big", bufs=1) as big,
            tc.tile_pool(name="work", bufs=2) as work,
            tc.tile_pool(name="scratch", bufs=1) as scratch,
            tc.tile_pool(name="psacc", bufs=4, space="PSUM") as psacc,
            tc.tile_pool(name="psmm", bufs=4, space="PSUM") as psmm,
        ):
            # ---------------- constants ----------------
            fwT_sb = const.tile([128, CC, C], F32R)
            nc.sync.dma_start(out=fwT_sb, in_=chunked(fwT.ap()).bitcast(F32R))
            # g_w and h_w are only needed in their (sequential) projection
            # phases — share one SBUF tile, reloading h_w over g_w.
            projw_sb = const.tile([128, CC, C], F32R)
            nc.sync.dma_start(out=projw_sb, in_=chunked(gwT.ap()).bitcast(F32R))
            fb_sb = const.tile([128, CC, 1], F32)
            gb_sb = const.tile([128, CC, 1], F32)
            hb_sb = const.tile([128, CC, 1], F32)
            nc.sync.dma_start(out=fb_sb, in_=chunked(fb.ap()))
            nc.sync.dma_start(out=gb_sb, in_=chunked(gb.ap()))
            nc.sync.dma_start(out=hb_sb, in_=chunked(hb.ap()))
            negshift = const.tile([128, 1], F32)
            nc.vector.memset(negshift, -SHIFT)
            ones_f = const.tile([128, 1], F32)
            nc.vector.memset(ones_f, 1.0)
            cmean = const.tile([128, CC, 1], F32)
            crstd = const.tile([128, CC, 1], F32)
            crstd2 = const.tile([128, CC, 1], F32)

            # ---------------- G = g_w @ style_key (f32r) -> fp16 ----------------
            Gfp = big.tile([128, CC, HW], F16)
            sk_ch = chunked(sk.ap()).bitcast(F32R)
            for nb in range(HW // 512):
                sk_t = stage.tile([128, CC, 512], F32R, tag="stage")
                nc.sync.dma_start(out=sk_t, in_=sk_ch[:, :, ts(nb, 512)])
                for co in range(CC):
                    gps = psmm.tile([128, 512], F32, tag="mm")
                    for ci in range(CC):
                        nc.tensor.matmul(
                            gps,
                            projw_sb[:, ci, ts(co, 128)],
                            sk_t[:, ci, :],
                            start=(ci == 0),
                            stop=(ci == CC - 1),
                        )
                    nc.vector.tensor_scalar_add(
                        Gfp[:, co, ts(nb, 512)], gps, gb_sb[:, co, :]
                    )

            # ---------------- HT[k, c] = (h_w @ style).T (f32r) -> fp16 ----------------
            HTF = big.tile([128, NKT, C], F16)
            nc.sync.dma_start(out=projw_sb, in_=chunked(hwT.ap()).bitcast(F32R))
            st_ch = chunked(st.ap()).bitcast(F32R)
            for nb in range(HW // 512):
                st_t = stage.tile([128, CC, 512], F32R, tag="stage")
                nc.sync.dma_start(out=st_t, in_=st_ch[:, :, ts(nb, 512)])
                for w in range(4):
                    kt = nb * 4 + w
                    hps = psmm.tile([128, 512], F32, tag="mm")
                    for ci in range(CC):
                        nc.tensor.matmul(
                            hps,
                            st_t[:, ci, ts(w, 128)],
                            projw_sb[:, ci, :],
                            start=(ci == 0),
                            stop=(ci == CC - 1),
                        )
                    nc.scalar.activation(
                        HTF[:, kt, :], hps, AF.Copy, bias=0.0, scale=1.0
                    )

            # -------- h2a = bf16(HTF^2) hoisted; h2b derived per kt in pass B ----
            h2a = big.tile([128, NKT, C], BF16)
            for kt in range(NKT):
                nc.scalar.activation(
                    h2a[:, kt, :], HTF[:, kt, :], AF.Square, bias=0.0, scale=1.0
                )

            # ---------------- main loop over query blocks ----------------
            ckq_ch = chunked(ckq.ap()).bitcast(F32R)
            ctq_ch = chunked(ctq.ap())
            out_ch = chunked(out.ap())
            ct_ch = chunked(ct.ap())
            stats_all = scratch.tile([128, 4, 8, 6], F32, tag="bnstats")
            pblk = big.tile([128, NKT, QB], BF16)

            def emit_stats_piece(i):
                # piece i: cc = i // 4, quarter = i % 4  -> one DMA + 2 bn_stats
                cc, quart = i // 4, i % 4
                ctp = stage.tile(
                    [128, 2, 512], F32, tag="stage", name=f"ctp{i}"
                )
                nc.sync.dma_start(
                    out=ctp,
                    in_=ct_ch[:, cc, ts(quart, 1024)].rearrange(
                        "p (a m) -> p a m", a=2
                    ),
                )
                for g in range(2):
                    nc.vector.bn_stats(
                        out=stats_all[:, cc, quart * 2 + g, :], in_=ctp[:, g, :]
                    )

            def emit_stats_tail():
                for cc in range(CC):
                    mv = scratch.tile([128, 2], F32, tag="bnmv")
                    nc.vector.bn_aggr(
                        out=mv,
                        in_=stats_all[:, cc, :, :].rearrange("p a b -> p (a b)"),
                    )
                    nc.vector.tensor_copy(cmean[:, cc, :], mv[:, 0:1])
                    tv = scratch.tile([128, 1], F32, tag="bntv")
                    nc.vector.tensor_scalar(
                        out=tv,
                        in0=mv[:, 1:2],
                        scalar1=float(HW) / float(HW - 1),
                        scalar2=EPS,
                        op0=ALU.mult,
                        op1=ALU.add,
                    )
                    sq = scratch.tile([128, 1], F32, tag="bnsq")
                    nc.scalar.activation(sq, tv, AF.Sqrt, bias=0.0, scale=1.0)
                    nc.vector.reciprocal(crstd[:, cc, :], sq)
                    nc.vector.tensor_mul(
                        crstd2[:, cc, :], crstd[:, cc, :], crstd[:, cc, :]
                    )

            for blk in range(NBLK):
                # F block (f32r matmul) -> fp16
                Ffp = work.tile([128, CC, QB], F16, tag="ffp")
                for hh in range(2):
                    qoff = blk * QB + hh * QH
                    ckq_t = stage.tile([128, CC, QH], F32R, tag="stage")
                    nc.sync.dma_start(
                        out=ckq_t, in_=ckq_ch[:, :, qoff : qoff + QH]
                    )
                    hs = slice(hh * QH, (hh + 1) * QH)
                    for co in range(CC):
                        fps = psmm.tile([128, QH], F32, tag="mm")
                        for ci in range(CC):
                            nc.tensor.matmul(
                                fps,
                                fwT_sb[:, ci, ts(co, 128)],
                                ckq_t[:, ci, :],
                                start=(ci == 0),
                                stop=(ci == CC - 1),
                            )
                        nc.vector.tensor_scalar_add(
                            Ffp[:, co, hs], fps, fb_sb[:, co, :]
                        )

                mean_ps = [
                    psacc.tile([128, QB], F32, tag="acc", name=f"mean{i}")
                    for i in range(CC)
                ]
                l_part = work.tile([128, QB], F32, tag="lpart", bufs=1)

                # ---- pass A: S -> P (bf16, stored) ; accumulate mean + l ----
                for kt in range(NKT):
                    sps = psmm.tile([128, QB], F32, tag="mm", name=f"sps{kt}")
                    for ci in range(CC):
                        nc.tensor.matmul(
                            sps,
                            Gfp[:, ci, ts(kt, 128)],
                            Ffp[:, ci, :],
                            start=(ci == 0),
                            stop=(ci == CC - 1),
                        )
                    nc.scalar.activation(
                        pblk[:, kt, :], sps, AF.Exp, bias=negshift, scale=1.0
                    )
                    if kt == 0:
                        nc.vector.tensor_copy(l_part, pblk[:, kt, :])
                    else:
                        nc.vector.tensor_add(l_part, l_part, pblk[:, kt, :])
                    for cc in range(CC):
                        nc.tensor.matmul(
                            mean_ps[cc],
                            HTF[:, kt, ts(cc, 128)],
                            pblk[:, kt, :],
                            start=(kt == 0),
                            stop=(kt == NKT - 1),
                        )
                    if blk == 0 and kt < 16:
                        emit_stats_piece(kt)
                if blk == 0:
                    emit_stats_tail()

                l_ps = psmm.tile([1, QB], F32, tag="mm", name="lps")
                nc.tensor.matmul(l_ps, ones_f, l_part, start=True, stop=True)
                rinv = scratch.tile([1, QB], F32, tag="ptmp")
                nc.vector.reciprocal(rinv, l_ps)
                rbc = scratch.tile([128, QB], F32, tag="rbc")
                nc.gpsimd.partition_broadcast(rbc, rinv[:1, :])

                # fold mean accumulators to SBUF (ACT) to free PSUM for pass B
                macc = work.tile([128, CC, QB], F32, tag="macc", bufs=1)
                for cc in range(CC):
                    nc.scalar.activation(
                        macc[:, cc, :], mean_ps[cc], AF.Copy, bias=0.0, scale=1.0
                    )

                # ---- pass B: accumulate second moment from stored P ----
                sec_ps = [
                    psacc.tile([128, QB], F32, tag="acc", name=f"sec{i}")
                    for i in range(CC)
                ]
                for kt in range(NKT):
                    h2f = work.tile([128, C], F32, tag="h2f", name=f"h2f{blk}_{kt}")
                    nc.scalar.activation(
                        h2f, HTF[:, kt, :], AF.Square, bias=0.0, scale=1.0
                    )
                    h2bt = work.tile([128, C], BF16, tag="h2bt", name=f"h2bt{blk}_{kt}")
                    nc.vector.tensor_sub(h2bt, h2f, h2a[:, kt, :])
                    for cc in range(CC):
                        nc.tensor.matmul(
                            sec_ps[cc],
                            h2a[:, kt, ts(cc, 128)],
                            pblk[:, kt, :],
                            start=(kt == 0),
                            stop=False,
                        )
                        nc.tensor.matmul(
                            sec_ps[cc],
                            h2bt[:, ts(cc, 128)],
                            pblk[:, kt, :],
                            start=False,
                            stop=(kt == NKT - 1),
                        )

                # ---- post: variance, std, assemble output ----
                ct_p0 = stage.tile([128, CC, QH], F32, tag="stage", name="ctp0")
                nc.sync.dma_start(
                    out=ct_p0, in_=ctq_ch[:, :, blk * QB : blk * QB + QH]
                )
                ct_p1 = stage.tile([128, CC, QH], F32, tag="stage", name="ctp1")
                nc.sync.dma_start(
                    out=ct_p1, in_=ctq_ch[:, :, blk * QB + QH : (blk + 1) * QB]
                )
                for cc in range(CC):
                    out_sb = work.tile([128, QB], F32, tag="outb")
                    mnp_t = work.tile([128, QB], F32, tag="lpart", bufs=1, name=f"mnpt{cc}")
                    nc.vector.tensor_mul(mnp_t, macc[:, cc, :], rbc)
                    e2 = scratch.tile([128, QB], F32, tag="ptmp")
                    nc.vector.tensor_mul(e2, sec_ps[cc], rbc)
                    msq = work.tile([128, QB], F32, tag="outb", name=f"msq{cc}")
                    nc.scalar.activation(msq, mnp_t, AF.Square, bias=0.0, scale=1.0)
                    var = work.tile([128, QB], F32, tag="ptf")
                    nc.vector.tensor_sub(var, e2, msq)
                    vmx = scratch.tile([128, QB], F32, tag="ptmp")
                    nc.scalar.activation(vmx, var, AF.Relu, bias=0.0, scale=1.0)
                    stdt = work.tile([128, QB], F32, tag="ptf", name=f"stdt{cc}")
                    nc.scalar.activation(
                        stdt, vmx, AF.Sqrt, bias=0.0, scale=crstd2[:, cc, :]
                    )
                    o1 = scratch.tile([128, QB], F32, tag="ptmp")
                    for hh, ctp in ((0, ct_p0), (1, ct_p1)):
                        nc.vector.scalar_tensor_tensor(
                            out=o1[:, ts(hh, QH)],
                            in0=ctp[:, cc, :],
                            scalar=cmean[:, cc, :],
                            in1=stdt[:, ts(hh, QH)],
                            op0=ALU.subtract,
                            op1=ALU.mult,
                        )
                    nc.vector.scalar_tensor_tensor(
                        out=out_sb,
                        in0=mnp_t,
                        scalar=hb_sb[:, cc, :],
                        in1=o1,
                        op0=ALU.add,
                        op1=ALU.add,
                    )
                    nc.sync.dma_start(
                        out=out_ch[:, cc, ts(blk, QB)], in_=out_sb
                    )

    nc.compile()
    return nc


_NC_CACHE = []


def kernel(content, style, content_key, style_key, f_w, f_b, g_w, g_b, h_w, h_b):
    if not _NC_CACHE:
        _NC_CACHE.append(_build())
    nc = _NC_CACHE[0]

    c32 = lambda a: np.ascontiguousarray(a, dtype=np.float32)

    fwT = c32(f_w.T)
    gwT = c32(g_w.T)
    hwT = c32(h_w.T)
    fbr = c32(np.asarray(f_b).reshape(C, 1))
    gbr = c32(np.asarray(g_b).reshape(C, 1))
    hbr = c32(np.asarray(h_b).reshape(C, 1))

    in_maps = []
    for core in range(8):
        b, h = core // 2, core % 2
        qsl = slice(h * Q, (h + 1) * Q)
        in_maps.append(
            {
                "ckq": c32(np.asarray(content_key[b]).reshape(C, HW)[:, qsl]),
                "sk": c32(np.asarray(style_key[b]).reshape(C, HW)),
                "st": c32(np.asarray(style[b]).reshape(C, HW)),
                "ct": c32(np.asarray(content[b]).reshape(C, HW)),
                "ctq": c32(np.asarray(content[b]).reshape(C, HW)[:, qsl]),
                "fwT": fwT,
                "gwT": gwT,
                "hwT": hwT,
                "fb": fbr,
                "gb": gbr,
                "hb": hbr,
            }
        )

    res = run_bass_kernel_spmd(nc, in_maps, core_ids=list(range(8)), trace=True)
    kernel.last_exec_time_ns = res.exec_time_ns

    full = np.empty((B, C, HW), dtype=np.float32)
    for core in range(8):
        b, h = core // 2, core % 2
        full[b][:, h * Q : (h + 1) * Q] = res.results[core]["out"]
    return full.reshape(B, C, 64, 64)


kernel.last_exec_time_ns = None


# revision 12
# speedup vs baseline: 1.8633x; 1.3220x over previous
"""AdaAttN Trainium2 kernel — 8-core SPMD, data-parallel over (batch, query-half).

Each core handles one (batch b, query half): 2048 of the 4096 query positions.
Single-matmul precision strategy (no bf16 two-term splits): the tensor engine
runs fp32r (moving free-dim >= 256) and fp16 matmuls at the same 1 cycle/row
rate as bf16, with ~11-bit-mantissa operand precision (FP22 internal), so:

  F  = f_w @ content_key[b][:, q]   [ck, q]  f32r matmul -> fp16
  G  = g_w @ style_key[b]           [ck, k]  f32r matmul -> fp16
  HT = (h_w @ style[b]).T           [k, c]   f32r matmul -> fp16 (HTF)
  S^T[k, q] = G.T @ F                        fp16 x fp16 matmul (4 MMs/kt)
  P = exp(S^T - 120) -> bf16 (pblk), stored for the whole query block

Consistency discipline for the variance: the bf16 P values are the single
source of truth — the normalizer l = sum_k P (DVE, from the same bf16
values), mean = HTF.T @ P, second = (HTF^2).T @ P with HTF^2 applied as an
exact bf16 pair (h2a stored + h2b derived per tile).  Then
second/l - (mean/l)^2 is the exact variance of quantized values under a
genuine probability distribution: nonnegative, no catastrophic-cancellation
amplification of quantization noise.

Pipelining for a gap-free PE stream (HAM stays warm):
  pass A per kt: S(kt) MMs, exp->pblk, l add, and mean MMs for kt-1 (the lag
  hides the ACT exp latency); pass B per kt: 8 second-moment MMs with the
  h2 square/split produced two tiles ahead; next block's F projection is
  emitted inside pass B; next block's S stream starts immediately after
  pass B while the post-processing (per-cc chains, DVE/ACT only) drains
  underneath — each sec PSUM bank is freed by its e2 read at the head of
  the per-cc chain, unblocking next block's mean accumulation.
ACT engine runs one function per phase (Exp / Square / Sqrt) to avoid
table-reload thrash; evacuations and relu/square post ops live on DVE.
PSUM: 4 banks ping-pong mean->second (psacc), 4 banks for the S ring and
projections (psmm).  h_b is folded into the final add (variance is
shift-invariant); f_b/g_b are added at F/G PSUM evacuation.

  out = sqrt(relu(second/l - (mean/l)^2)) * mvnorm(content) + mean/l + h_b
"""

import numpy as np

import concourse.bass as bass
import concourse.mybir as mybir
from concourse import bacc
from concourse.bass import ts
from concourse.bass_utils import run_bass_kernel_spmd
from concourse.tile import TileContext

F32 = mybir.dt.float32
F32R = mybir.dt.float32r
F16 = mybir.dt.float16
BF16 = mybir.dt.bfloat16
AF = mybir.ActivationFunctionType
ALU = mybir.AluOpType

B, C, HW = 4, 512, 4096  # batch, channels (=key planes), spatial
Q = 2048                 # queries per core (half a batch)
QB = 512                 # query block
QH = 256                 # half-block (DMA/staging granularity)
NBLK = Q // QB           # 4
CC = C // 128            # 4 channel chunks
NKT = HW // 128          # 32 key tiles
SHIFT = 120.0
EPS = 1e-5


def _build():
    nc = bacc.Bacc("TRN2", target_bir_lowering=False, debug=False)

    ckq = nc.declare_dram_parameter("ckq", [C, Q], F32, isOutput=False)
    sk = nc.declare_dram_parameter("sk", [C, HW], F32, isOutput=False)
    st = nc.declare_dram_parameter("st", [C, HW], F32, isOutput=False)
    ct = nc.declare_dram_parameter("ct", [C, HW], F32, isOutput=False)
    ctq = nc.declare_dram_parameter("ctq", [C, Q], F32, isOutput=False)
    fwT = nc.declare_dram_parameter("fwT", [C, C], F32, isOutput=False)
    gwT = nc.declare_dram_parameter("gwT", [C, C], F32, isOutput=False)
    hwT = nc.declare_dram_parameter("hwT", [C, C], F32, isOutput=False)
    fb = nc.declare_dram_parameter("fb", [C, 1], F32, isOutput=False)
    gb = nc.declare_dram_parameter("gb", [C, 1], F32, isOutput=False)
    hb = nc.declare_dram_parameter("hb", [C, 1], F32, isOutput=False)
    out = nc.declare_dram_parameter("out", [C, Q], F32, isOutput=True)

    # [512, M] dram -> [128, 4, M] (partition = channel-within-chunk)
    def chunked(ap):
        return ap.rearrange("(a p) m -> p a m", p=128)

    with TileContext(nc) as tc:
        with (
            tc.tile_pool(name="const", bufs=1) as const,
            tc.tile_pool(name="stage", bufs=3) as stage,
            tc.tile_pool(name="big", bufs=1) as big,
            tc.tile_pool(name="work", bufs=2) as work,
            tc.tile_pool(name="scratch", bufs=1) as scratch,
            tc.tile_pool(name="psacc", bufs=4, space="PSUM") as psacc,
            tc.tile_pool(name="psmm", bufs=4, space="PSUM") as psmm,
        ):
            # ---------------- constants ----------------
            fwT_sb = const.tile([128, CC, C], F32R)
            nc.sync.dma_start(out=fwT_sb, in_=chunked(fwT.ap()).bitcast(F32R))
            # g_w and h_w are only needed in their (sequential) projection
            # phases — share one SBUF tile, reloading h_w over g_w.
            projw_sb = const.tile([128, CC, C], F32R)
            nc.sync.dma_start(out=projw_sb, in_=chunked(gwT.ap()).bitcast(F32R))
            fb_sb = const.tile([128, CC, 1], F32)
            gb_sb = const.tile([128, CC, 1], F32)
            hb_sb = const.tile([128, CC, 1], F32)
            nc.sync.dma_start(out=fb_sb, in_=chunked(fb.ap()))
            nc.sync.dma_start(out=gb_sb, in_=chunked(gb.ap()))
            nc.sync.dma_start(out=hb_sb, in_=chunked(hb.ap()))
            negshift = const.tile([128, 1], F32)
            nc.vector.memset(negshift, -SHIFT)
            ones_f = const.tile([128, 1], F32)
            nc.vector.memset(ones_f, 1.0)
            cmean = const.tile([128, CC, 1], F32)
            crstd = const.tile([128, CC, 1], F32)
            crstd2 = const.tile([128, CC, 1], F32)

            # ------------- G = g_w @ style_key (f32r) -> fp16 (DVE evac) -------
            Gfp = big.tile([128, CC, HW], F16)
            sk_ch = chunked(sk.ap()).bitcast(F32R)
            for nb in range(HW // 256):
                sk_t = stage.tile([128, CC, 256], F32R, tag="ld4")
                nc.sync.dma_start(out=sk_t, in_=sk_ch[:, :, ts(nb, 256)])
                for co in range(CC):
                    gps = psmm.tile([128, 256], F32, tag="mm")
                    for ci in range(CC):
                        nc.tensor.matmul(
                            gps,
                            projw_sb[:, ci, ts(co, 128)],
                            sk_t[:, ci, :],
                            start=(ci == 0),
                            stop=(ci == CC - 1),
                        )
                    nc.vector.tensor_scalar_add(
                        Gfp[:, co, ts(nb, 256)], gps, gb_sb[:, co, :]
                    )

            # ------- HT[k, c] = (h_w @ style).T (f32r) -> fp16; h2a = bf16(HT^2)
            # (HT evac on DVE; ACT runs only Square in this phase)
            HTF = big.tile([128, NKT, C], F16)
            h2a = big.tile([128, NKT, C], BF16)
            nc.sync.dma_start(out=projw_sb, in_=chunked(hwT.ap()).bitcast(F32R))
            st_ch = chunked(st.ap()).bitcast(F32R)
            for nb in range(HW // 256):
                st_t = stage.tile([128, CC, 256], F32R, tag="ld4")
                nc.sync.dma_start(out=st_t, in_=st_ch[:, :, ts(nb, 256)])
                for w in range(2):
                    kt = nb * 2 + w
                    hps = psmm.tile([128, 512], F32, tag="mm")
                    for ci in range(CC):
                        nc.tensor.matmul(
                            hps,
                            st_t[:, ci, ts(w, 128)],
                            projw_sb[:, ci, :],
                            start=(ci == 0),
                            stop=(ci == CC - 1),
                        )
                    nc.vector.tensor_copy(HTF[:, kt, :], hps)
                    nc.scalar.activation(
                        h2a[:, kt, :], HTF[:, kt, :], AF.Square, bias=0.0,
                        scale=1.0,
                    )

            # ---------------- main loop over query blocks ----------------
            ckq_ch = chunked(ckq.ap()).bitcast(F32R)
            ctq_ch = chunked(ctq.ap())
            out_ch = chunked(out.ap())
            ct_ch = chunked(ct.ap())
            stats_all = scratch.tile([128, 4, 8, 6], F32, tag="bnstats")
            pblk = big.tile([128, NKT, QB], BF16)

            def emit_F(blk):
                Ffp = work.tile(
                    [128, CC, QB], F16, tag="ffp", name=f"ffp{blk}"
                )
                for hh in range(2):
                    qoff = blk * QB + hh * QH
                    ckq_t = stage.tile(
                        [128, CC, QH], F32R, tag="ld4", name=f"ckq{blk}_{hh}"
                    )
                    nc.sync.dma_start(
                        out=ckq_t, in_=ckq_ch[:, :, qoff : qoff + QH]
                    )
                    hs = slice(hh * QH, (hh + 1) * QH)
                    for co in range(CC):
                        fps = psmm.tile([128, QH], F32, tag="mm")
                        for ci in range(CC):
                            nc.tensor.matmul(
                                fps,
                                fwT_sb[:, ci, ts(co, 128)],
                                ckq_t[:, ci, :],
                                start=(ci == 0),
                                stop=(ci == CC - 1),
                            )
                        nc.vector.tensor_scalar_add(
                            Ffp[:, co, hs], fps, fb_sb[:, co, :]
                        )
                return Ffp

            def emit_stats_piece(i):
                # piece i: cc = i // 4, quarter = i % 4  -> one DMA + 2 bn_stats
                cc, quart = i // 4, i % 4
                ctp = stage.tile(
                    [128, 4, 256], F32, tag="ld4", name=f"ctp{i}"
                )
                nc.sync.dma_start(
                    out=ctp,
                    in_=ct_ch[:, cc, ts(quart, 1024)].rearrange(
                        "p (a m) -> p a m", a=4
                    ),
                )
                flat = ctp.rearrange("p a m -> p (a m)")
                for g in range(2):
                    nc.vector.bn_stats(
                        out=stats_all[:, cc, quart * 2 + g, :],
                        in_=flat[:, ts(g, 512)],
                    )

            def emit_stats_tail():
                for cc in range(CC):
                    mv = scratch.tile([128, 2], F32, tag="bnmv")
                    nc.vector.bn_aggr(
                        out=mv,
                        in_=stats_all[:, cc, :, :].rearrange("p a b -> p (a b)"),
                    )
                    nc.vector.tensor_copy(cmean[:, cc, :], mv[:, 0:1])
                    tv = scratch.tile([128, 1], F32, tag="bntv")
                    nc.vector.tensor_scalar(
                        out=tv,
                        in0=mv[:, 1:2],
                        scalar1=float(HW) / float(HW - 1),
                        scalar2=EPS,
                        op0=ALU.mult,
                        op1=ALU.add,
                    )
                    sq = scratch.tile([128, 1], F32, tag="bnsq")
                    nc.scalar.activation(sq, tv, AF.Sqrt, bias=0.0, scale=1.0)
                    nc.vector.reciprocal(crstd[:, cc, :], sq)
                    nc.vector.tensor_mul(
                        crstd2[:, cc, :], crstd[:, cc, :], crstd[:, cc, :]
                    )

            Ffp = emit_F(0)
            for blk in range(NBLK):
                # ---- pass A: S -> P (bf16, stored); mean lags S by one kt ----
                mean_ps = [
                    psacc.tile([128, QB], F32, tag="acc", name=f"mean{blk}_{i}")
                    for i in range(CC)
                ]
                l_part = work.tile([128, QB], F32, tag="lpart", bufs=1)

                def emit_mean(kt):
                    for cc in range(CC):
                        nc.tensor.matmul(
                            mean_ps[cc],
                            HTF[:, kt, ts(cc, 128)],
                            pblk[:, kt, :],
                            start=(kt == 0),
                            stop=(kt == NKT - 1),
                        )

                for kt in range(NKT):
                    sps = psmm.tile(
                        [128, QB], F32, tag="mm", name=f"sps{blk}_{kt}"
                    )
                    for ci in range(CC):
                        nc.tensor.matmul(
                            sps,
                            Gfp[:, ci, ts(kt, 128)],
                            Ffp[:, ci, :],
                            start=(ci == 0),
                            stop=(ci == CC - 1),
                        )
                    nc.scalar.activation(
                        pblk[:, kt, :], sps, AF.Exp, bias=negshift, scale=1.0
                    )
                    if kt == 0:
                        nc.vector.tensor_copy(l_part, pblk[:, kt, :])
                    else:
                        nc.vector.tensor_add(l_part, l_part, pblk[:, kt, :])
                    if kt >= 1:
                        emit_mean(kt - 1)
                    if blk == 0 and kt % 4 == 0:
                        emit_stats_piece(kt // 4)
                emit_mean(NKT - 1)

                l_ps = psmm.tile([1, QB], F32, tag="mm", name=f"lps{blk}")
                nc.tensor.matmul(l_ps, ones_f, l_part, start=True, stop=True)
                rinv = scratch.tile([1, QB], F32, tag="rinv")
                nc.vector.reciprocal(rinv, l_ps)
                rbc = scratch.tile([128, QB], F32, tag="rbc")
                nc.gpsimd.partition_broadcast(rbc, rinv[:1, :])

                # fold mean accumulators to SBUF (DVE) to free PSUM for pass B
                macc = work.tile([128, CC, QB], F32, tag="macc", bufs=1)
                for cc in range(CC):
                    nc.vector.tensor_copy(macc[:, cc, :], mean_ps[cc])

                # ---- pass B: second moment from stored P; h2 split 2 kt ahead
                sec_ps = [
                    psacc.tile([128, QB], F32, tag="acc", name=f"sec{blk}_{i}")
                    for i in range(CC)
                ]
                h2bts = {}

                def emit_h2pipe(kt):
                    h2f = work.tile(
                        [128, C], F32, tag="h2f", name=f"h2f{blk}_{kt}"
                    )
                    nc.scalar.activation(
                        h2f, HTF[:, kt, :], AF.Square, bias=0.0, scale=1.0
                    )
                    h2bt = work.tile(
                        [128, C], BF16, tag="h2bt", bufs=3,
                        name=f"h2bt{blk}_{kt}",
                    )
                    nc.vector.tensor_sub(h2bt, h2f, h2a[:, kt, :])
                    h2bts[kt] = h2bt

                emit_h2pipe(0)
                emit_h2pipe(1)
                for kt in range(NKT):
                    if kt + 2 < NKT:
                        emit_h2pipe(kt + 2)
                    h2bt = h2bts.pop(kt)
                    for cc in range(CC):
                        nc.tensor.matmul(
                            sec_ps[cc],
                            h2a[:, kt, ts(cc, 128)],
                            pblk[:, kt, :],
                            start=(kt == 0),
                            stop=False,
                        )
                        nc.tensor.matmul(
                            sec_ps[cc],
                            h2bt[:, ts(cc, 128)],
                            pblk[:, kt, :],
                            start=False,
                            stop=(kt == NKT - 1),
                        )
                    if kt == 2 and blk + 1 < NBLK:
                        Ffp_next = emit_F(blk + 1)
                    if blk == 0 and kt % 4 == 1:
                        emit_stats_piece(8 + kt // 4)
                if blk == 0:
                    emit_stats_tail()

                # ---- post: variance, std, assemble output (DVE/ACT only;
                # e2 read at the head of each chain frees the sec bank) ----
                ct_p0 = stage.tile(
                    [128, CC, QH], F32, tag="ld4", name=f"ctq{blk}_0"
                )
                nc.sync.dma_start(
                    out=ct_p0, in_=ctq_ch[:, :, blk * QB : blk * QB + QH]
                )
                ct_p1 = stage.tile(
                    [128, CC, QH], F32, tag="ld4", name=f"ctq{blk}_1"
                )
                nc.sync.dma_start(
                    out=ct_p1, in_=ctq_ch[:, :, blk * QB + QH : (blk + 1) * QB]
                )
                for cc in range(CC):
                    e2 = scratch.tile(
                        [128, QB], F32, tag="ptmp", bufs=2, name=f"e2_{blk}{cc}"
                    )
                    nc.vector.tensor_mul(e2, sec_ps[cc], rbc)
                    mnp_t = work.tile(
                        [128, QB], F32, tag="mnp", name=f"mnpt{blk}{cc}"
                    )
                    nc.vector.tensor_mul(mnp_t, macc[:, cc, :], rbc)
                    msq = work.tile([128, QB], F32, tag="outb", name=f"msq{blk}{cc}")
                    nc.vector.tensor_mul(msq, mnp_t, mnp_t)
                    var = work.tile([128, QB], F32, tag="ptf", name=f"var{blk}{cc}")
                    nc.vector.tensor_sub(var, e2, msq)
                    vmx = scratch.tile(
                        [128, QB], F32, tag="ptmp", bufs=2, name=f"vmx{blk}{cc}"
                    )
                    nc.vector.tensor_scalar_max(vmx, var, 0.0)
                    stdt = work.tile([128, QB], F32, tag="ptf", name=f"stdt{blk}{cc}")
                    nc.scalar.activation(
                        stdt, vmx, AF.Sqrt, bias=0.0, scale=crstd2[:, cc, :]
                    )
                    o1 = scratch.tile(
                        [128, QB], F32, tag="po1", bufs=2, name=f"o1_{blk}{cc}"
                    )
                    for hh, ctp in ((0, ct_p0), (1, ct_p1)):
                        nc.vector.scalar_tensor_tensor(
                            out=o1[:, ts(hh, QH)],
                            in0=ctp[:, cc, :],
                            scalar=cmean[:, cc, :],
                            in1=stdt[:, ts(hh, QH)],
                            op0=ALU.subtract,
                            op1=ALU.mult,
                        )
                    out_sb = work.tile([128, QB], F32, tag="outb", name=f"ob{blk}{cc}")
                    nc.vector.scalar_tensor_tensor(
                        out=out_sb,
                        in0=mnp_t,
                        scalar=hb_sb[:, cc, :],
                        in1=o1,
                        op0=ALU.add,
                        op1=ALU.add,
                    )
                    nc.sync.dma_start(
                        out=out_ch[:, cc, ts(blk, QB)], in_=out_sb
                    )
                if blk + 1 < NBLK:
                    Ffp = Ffp_next

    nc.compile()
    return nc


_NC_CACHE = []


def kernel(content, style, content_key, style_key, f_w, f_b, g_w, g_b, h_w, h_b):
    if not _NC_CACHE:
        _NC_CACHE.append(_build())
    nc = _NC_CACHE[0]

    c32 = lambda a: np.ascontiguousarray(a, dtype=np.float32)

    fwT = c32(f_w.T)
    gwT = c32(g_w.T)
    hwT = c32(h_w.T)
    fbr = c32(np.asarray(f_b).reshape(C, 1))
    gbr = c32(np.asarray(g_b).reshape(C, 1))
    hbr = c32(np.asarray(h_b).reshape(C, 1))

    in_maps = []
    for core in range(8):
        b, h = core // 2, core % 2
        qsl = slice(h * Q, (h + 1) * Q)
        in_maps.append(
            {
                "ckq": c32(np.asarray(content_key[b]).reshape(C, HW)[:, qsl]),
                "sk": c32(np.asarray(style_key[b]).reshape(C, HW)),
                "st": c32(np.asarray(style[b]).reshape(C, HW)),
                "ct": c32(np.asarray(content[b]).reshape(C, HW)),
                "ctq": c32(np.asarray(content[b]).reshape(C, HW)[:, qsl]),
                "fwT": fwT,
                "gwT": gwT,
                "hwT": hwT,
                "fb": fbr,
                "gb": gbr,
                "hb": hbr,
            }
        )

    res = run_bass_kernel_spmd(nc, in_maps, core_ids=list(range(8)), trace=True)
    kernel.last_exec_time_ns = res.exec_time_ns

    full = np.empty((B, C, HW), dtype=np.float32)
    for core in range(8):
        b, h = core // 2, core % 2
        full[b][:, h * Q : (h + 1) * Q] = res.results[core]["out"]
    return full.reshape(B, C, 64, 64)


kernel.last_exec_time_ns = None
